# revision 25
# baseline (speedup 1.0000x reference)
"""Multi-head attention (bsz=2, seq=2048, hidden=1024, heads=16) on 8 TRN2 cores.

Sharding: core c = 4*b + g handles batch b and heads [4g, 4g+4).
Each core computes Q/K/V projections for its 4 heads, causal softmax
attention, and a partial output projection over its 256 features; the host
sums the 4 per-batch partials.

Redesign vs the 322us baseline (which ran the PE at cold clock):
- bf16 operands everywhere (same 1 cyc/row as f32r, half the DMA/SBUF/DVE
  cost, FWL-eligible weight loads); fp32 PSUM accumulation throughout.
- Host pre-tiles every DRAM tensor partition-major so each load is one large
  partition-contiguous DMA (the first x chunk is split per-slice so the
  first projection chain starts as slices land).
- Dummy warmup matmuls on the mask tile bridge the initial DMA window so the
  HAM clock gate reaches full clock before real work; the emission never
  leaves PE-idle gaps >3.4us, so it never re-throttles.
- Scores for a head PAIR run as two concurrent row-tiled matmuls (head-dim
  64 contraction at base partitions 0/64) into one 2-bank PSUM tile, so the
  exp covers [128, 2x512] in a single ACT instruction (amortizes the
  352-cycle ACT overhead).
- Softmax 1/denominator as exp(-ln(d)) on the scalar engine (the DVE
  reciprocal is ~6 cyc/elem and blocks the DVE FIFO); a pre-placed
  InstLoadActFuncSet pins the combined natural_log_exp_and_others table so
  the compiler never thrashes table sets.
- Software-pipelined emission: PV lags scores by one block, and projection /
  output-projection matmuls of neighboring chunks are pumped as fillers
  between attention blocks, paced by a pe-vs-act ns ledger, so the strict
  FIFO PE queue never waits on the scalar engine. The last chunk runs its
  head pairs in reverse, splits its O-projection per pair (host sums the
  extra partial), and normalizes per q-half so the tail pipelines.
"""

import sys

sys.path.insert(0, "/opt/trn_rl_repo")

from collections import deque
from contextlib import ExitStack

import numpy as np
import ml_dtypes

import concourse.tile as tile
from concourse import bacc, bass_utils, mybir

B, S, H = 2, 2048, 1024
NHC = 4  # heads per core
HD = 64  # head dim
F = NHC * HD  # features per core (256)
N_CORES = 8
QC = 512  # query-chunk width
KB = 128  # key-block size
SCALE = 1.0 / 8.0  # 1/sqrt(HD)
NWARM = 26

F32 = mybir.dt.float32
F32R = mybir.dt.float32r
BF16 = mybir.dt.bfloat16
EXP = mybir.ActivationFunctionType.Exp
LN = mybir.ActivationFunctionType.Ln

_CACHE = {}


def _emit(tc):
    nc = tc.nc
    xt_d = nc.dram_tensor("xt", [4, KB, 8, QC], BF16, kind="ExternalInput").ap()
    wq_d = nc.dram_tensor("wq", [KB, 8, F], BF16, kind="ExternalInput").ap()
    wk_d = nc.dram_tensor("wk", [KB, 8, F], BF16, kind="ExternalInput").ap()
    wv_d = nc.dram_tensor("wv", [KB, 8, F], BF16, kind="ExternalInput").ap()
    wo_d = nc.dram_tensor("wo", [KB, 2, 2 * QC], BF16, kind="ExternalInput").ap()
    mtri_d = nc.dram_tensor("mtri", [KB, KB], BF16, kind="ExternalInput").ap()
    out_d = nc.dram_tensor("out", [16, 2, KB, QC], BF16, kind="ExternalOutput").ap()
    # chunk-3 O-proj is split per head-pair so its fc1 half can run early;
    # the host sums this extra partial into out[12:16]
    out2_d = nc.dram_tensor("out2", [4, 2, KB, QC], BF16, kind="ExternalOutput").ap()

    ctx = tc._emit_ctx
    const = ctx.enter_context(tc.tile_pool(name="const", bufs=1))
    persist = ctx.enter_context(tc.tile_pool(name="persist", bufs=1))
    xpool = ctx.enter_context(tc.tile_pool(name="xp", bufs=2))
    ptp = ctx.enter_context(tc.tile_pool(name="ptp", bufs=4))
    rcp = ctx.enter_context(tc.tile_pool(name="rcp", bufs=2))
    ost = ctx.enter_context(tc.tile_pool(name="ost", bufs=6))
    ps_st = ctx.enter_context(tc.tile_pool(name="psst", bufs=2, space="PSUM"))
    ps_po = ctx.enter_context(tc.tile_pool(name="pspo", bufs=2, space="PSUM"))
    ps_pj = ctx.enter_context(tc.tile_pool(name="pspj", bufs=2, space="PSUM"))

    # ---- persistent SBUF tiles ----
    wq_s = persist.tile([KB, 8 * F], BF16, tag="wq_s")
    wk_s = persist.tile([KB, 8 * F], BF16, tag="wk_s")
    wv_s = persist.tile([KB, 8 * F], BF16, tag="wv_s")
    wo_s = persist.tile([KB, 4 * QC], BF16, tag="wo_s")
    mtri = const.tile([KB, KB], BF16, tag="mtri")
    qts = [persist.tile([KB, S], BF16, tag=f"qt{i}", name=f"qt{i}") for i in range(2)]
    kts = [persist.tile([KB, S], BF16, tag=f"kt{i}", name=f"kt{i}") for i in range(2)]
    ats = [persist.tile([KB, S], BF16, tag=f"at{i}", name=f"at{i}") for i in range(2)]
    # V_aug per k-block: per head [V_h (64) | ones (64)] -> [128, 512]
    vts = [
        persist.tile([KB, NHC * 2 * HD], BF16, tag=f"vt{i}", name=f"vt{i}")
        for i in range(16)
    ]

    # ---- initial DMAs, priority order (Q-proj needs wq+x0 first) ----
    nc.sync.dma_start(mtri[:], mtri_d[:])
    nc.sync.dma_start(wq_s[:].rearrange("p (h c) -> p h c", h=8), wq_d[:])
    xall = [None] * 4
    xall[0] = xpool.tile([KB, 8 * QC], BF16, tag="x", name="x0")
    for hc in range(8):
        nc.sync.dma_start(
            xall[0][:, hc * QC : (hc + 1) * QC], xt_d[0, :, hc]
        )
    nc.sync.dma_start(wk_s[:].rearrange("p (h c) -> p h c", h=8), wk_d[:])
    nc.sync.dma_start(wv_s[:].rearrange("p (h c) -> p h c", h=8), wv_d[:])
    nc.sync.dma_start(wo_s[:].rearrange("p (h c) -> p h c", h=2), wo_d[:])

    # ---- memsets (DVE) ----
    for rc in range(16):
        v4 = vts[rc][:].rearrange("p (h d) -> p h d", h=NHC)
        nc.vector.memset(v4[:, :, HD : 2 * HD], 1.0)

    # ---- PE warmup (bridges the initial DMA window; gets HAM warm) ----
    for i in range(NWARM):
        ps = ps_pj.tile([KB, QC], F32, tag="pj", name=f"warm{i}")
        nc.tensor.matmul(ps[:, :KB], mtri[:], mtri[:], start=True, stop=True)
    # pre-place the combined ln+exp table set (natural_log_exp_and_others,
    # id 6) so the fixpoint never thrashes between exp_and_others and
    # natural_log at every softmax normalize
    nc.scalar.add_instruction(
        mybir.InstLoadActFuncSet(
            name=nc.get_next_instruction_name(), act_func_set_id=6, ins=[], outs=[]
        )
    )
    warm_pt = const.tile([1, 4], BF16, tag="warm_pt")
    nc.scalar.activation(warm_pt[:], mtri[0:1, 0:4], EXP, scale=SCALE)

    # ---- pacing ledger for filler pumping ----
    led = {"pe": 0.0, "act": 0.0}
    fillers = deque()
    vgen = {}  # rc -> generator writing vts[rc], for targeted force-drain

    def pump_one():
        while fillers:
            gen = fillers[0]
            try:
                led["pe"] += next(gen)
                return True
            except StopIteration:
                fillers.popleft()
        return False

    def pump():
        while led["act"] > led["pe"]:
            if not pump_one():
                return

    def drain_all():
        while pump_one():
            pass

    def drain_gen(gen):
        for _ in gen:
            pass

    # ---- projection chain generators (each yield = one PE matmul emitted) ----
    def gen_qk(jq, w_s, dst, what):
        xa = xall[jq]
        q0 = jq * QC
        for fc in range(2):
            ps = ps_pj.tile([KB, QC], F32, tag="pj", name=f"p{what}{jq}_{fc}")
            for hc in range(8):
                nc.tensor.matmul(
                    ps[:],
                    w_s[:, hc * F + fc * KB : hc * F + (fc + 1) * KB],
                    xa[:, hc * QC : (hc + 1) * QC],
                    start=(hc == 0),
                    stop=(hc == 7),
                )
                yield 223
            nc.vector.tensor_copy(dst[fc][:, q0 : q0 + QC], ps[:])

    def gen_v(jq):
        xa = xall[jq]
        for sub in range(4):
            rc = 4 * jq + sub
            ps = ps_pj.tile([KB, QC], F32, tag="pj", name=f"pv{rc}")
            for hc in range(8):
                nc.tensor.matmul(
                    ps[:, :F],
                    xa[:, hc * QC + sub * KB : hc * QC + (sub + 1) * KB],
                    wv_s[:, hc * F : (hc + 1) * F],
                    start=(hc == 0),
                    stop=(hc == 7),
                )
                yield 117
            v4 = vts[rc][:].rearrange("p (h d) -> p h d", h=NHC)
            nc.vector.tensor_copy(
                v4[:, :, 0:HD], ps[:, :F].rearrange("p (h d) -> p h d", h=NHC)
            )
            vgen.pop(rc, None)

    def make_v_tracked(jq):
        # split per-sub and register eagerly so pv_block can force-drain
        # exactly the tile it needs even before the pump reaches it
        gens = []
        for sub in range(4):
            rc = 4 * jq + sub
            g = _gen_v_one(jq, sub, rc)
            vgen[rc] = g
            gens.append(g)

        def runner():
            for g in gens:
                yield from g

        return runner()

    def _gen_v_one(jq, sub, rc):
        xa = xall[jq]
        ps = ps_pj.tile([KB, QC], F32, tag="pj", name=f"pv{rc}")
        for hc in range(8):
            nc.tensor.matmul(
                ps[:, :F],
                xa[:, hc * QC + sub * KB : hc * QC + (sub + 1) * KB],
                wv_s[:, hc * F : (hc + 1) * F],
                start=(hc == 0),
                stop=(hc == 7),
            )
            yield 117
        v4 = vts[rc][:].rearrange("p (h d) -> p h d", h=NHC)
        nc.vector.tensor_copy(
            v4[:, :, 0:HD], ps[:, :F].rearrange("p (h d) -> p h d", h=NHC)
        )
        vgen.pop(rc, None)

    def gen_oproj(jq):
        for sub in range(4):
            qb = 4 * jq + sub
            for oc in range(2):
                ps = ps_pj.tile([KB, QC], F32, tag="pj", name=f"po{qb}_{oc}")
                for fc in range(2):
                    nc.tensor.matmul(
                        ps[:],
                        ats[fc][:, qb * KB : (qb + 1) * KB],
                        wo_s[:, fc * 2 * QC + oc * QC : fc * 2 * QC + (oc + 1) * QC],
                        start=(fc == 0),
                        stop=(fc == 1),
                    )
                    yield 223
                o = ost.tile([KB, QC], BF16, tag="ost", name=f"os{qb}_{oc}")
                nc.vector.tensor_copy(o[:], ps[:])
                nc.sync.dma_start(out_d[qb, oc], o[:])

    def gen_oproj3_fc(fc):
        # chunk-3 O-proj, one head-pair at a time: single-MM partials so the
        # fc1 half runs as soon as pair 1 is normalized (host sums partials)
        for sub in range(4):
            qb = 12 + sub
            for oc in range(2):
                ps = ps_pj.tile([KB, QC], F32, tag="pj", name=f"po3{fc}_{qb}_{oc}")
                nc.tensor.matmul(
                    ps[:],
                    ats[fc][:, qb * KB : (qb + 1) * KB],
                    wo_s[:, fc * 2 * QC + oc * QC : fc * 2 * QC + (oc + 1) * QC],
                    start=True,
                    stop=True,
                )
                yield 223
                o = ost.tile([KB, QC], BF16, tag="ost", name=f"o3{fc}_{qb}_{oc}")
                nc.vector.tensor_copy(o[:], ps[:])
                if fc == 0:
                    nc.sync.dma_start(out_d[qb, oc], o[:])
                else:
                    nc.sync.dma_start(out2_d[sub, oc], o[:])

    # ---- attention for one chunk (both head pairs), filler-interleaved ----
    def attention(jq):
        q0 = jq * QC
        nkb = 4 * jq + 4
        # last chunk: pair 1 first so its O-proj half overlaps pair 0
        pair_order = (1, 0) if jq == 3 else (0, 1)
        for t in pair_order:
            po = [
                ps_po.tile([KB, QC], F32, tag="po", name=f"pp{jq}_{t}_{h}")
                for h in range(2)
            ]
            pts = {}

            def s_block(ik):
                r = ik - 4 * jq
                qk = 0 if r <= 0 else r * KB
                ex = qk
                st = ps_st.tile([KB, 2 * QC], F32, tag="st", name=f"st{jq}_{t}_{ik}")
                nc.tensor.matmul(
                    st[:, qk:QC],
                    kts[t][0:HD, ik * KB : (ik + 1) * KB],
                    qts[t][0:HD, q0 + qk : q0 + QC],
                    start=True,
                    stop=True,
                )
                nc.tensor.matmul(
                    st[:, QC + qk : 2 * QC],
                    kts[t][HD : 2 * HD, ik * KB : (ik + 1) * KB],
                    qts[t][HD : 2 * HD, q0 + qk : q0 + QC],
                    start=True,
                    stop=True,
                )
                led["pe"] += (QC - qk) / 2.4 + 20
                pt = ptp.tile([KB, 2 * QC], BF16, tag="pt", name=f"pt{jq}_{t}_{ik}")
                stv = st[:].rearrange("p (h c) -> p h c", h=2)
                ptv = pt[:].rearrange("p (h c) -> p h c", h=2)
                nc.scalar.activation(
                    ptv[:, :, ex:QC], stv[:, :, ex:QC], EXP, scale=SCALE
                )
                led["act"] += (2 * (QC - ex) + 352) / 1.2
                if r >= 0:
                    trv = ptv[:, :, r * KB : (r + 1) * KB]
                    mb = mtri[:].rearrange("p c -> p () c").broadcast_to(
                        (KB, 2, KB)
                    )
                    nc.vector.tensor_mul(trv, trv, mb)
                pts[ik] = (pt, qk)

            def pv_block(ik):
                if ik in vgen:  # guarantee vts[ik] is emitted before its reader
                    drain_gen(vgen.pop(ik))
                pt, qk = pts.pop(ik)
                for h in range(2):
                    hh = 2 * t + h
                    nc.tensor.matmul(
                        po[h][:, qk:QC],
                        vts[ik][:, hh * 2 * HD : (hh + 1) * 2 * HD],
                        pt[:, h * QC + qk : h * QC + QC],
                        start=(ik == 0),
                        stop=(ik == nkb - 1),
                    )
                led["pe"] += 2 * (QC - qk) / 2.4 + 40

            for ik in range(nkb):
                s_block(ik)
                # pump between S and PV: fillers absorb the tail of
                # exp(ik-1) that PV(ik-1) would otherwise stall on
                pump()
                if ik >= 1:
                    pv_block(ik - 1)
            pv_block(nkb - 1)
            # normalize: partitions 64:128 of po hold the denominator.
            # 1/d as exp(-ln(d)) on the scalar engine (pipelined; the DVE
            # iterative reciprocal is ~6 cyc/elem and stalls the pipeline).
            # ln+exp share one table set (natural_log_exp_and_others).
            if jq == 3 and t == pair_order[1]:
                # final pair: normalize per q-half, interleaving the fc0
                # O-proj singles so the tail chain pipelines
                g0 = gen_oproj3_fc(0)
                for half in range(2):
                    c0, c1 = half * 256, (half + 1) * 256
                    for h in range(2):
                        lt = rcp.tile(
                            [HD, 256], F32, tag="lnh", name=f"lnh{half}_{h}"
                        )
                        rt = rcp.tile(
                            [HD, 256], F32, tag="rch", name=f"rch{half}_{h}"
                        )
                        nc.scalar.activation(lt[:], po[h][HD : 2 * HD, c0:c1], LN)
                        nc.scalar.activation(rt[:], lt[:], EXP, scale=-1.0)
                        nc.vector.tensor_mul(
                            ats[t][h * HD : (h + 1) * HD, q0 + c0 : q0 + c1],
                            po[h][0:HD, c0:c1],
                            rt[:],
                        )
                    for _ in range(4):  # subs {0,1} then {2,3}, 2 oc each
                        try:
                            next(g0)
                        except StopIteration:
                            break
                drain_gen(g0)
                return
            rts = []
            for h in range(2):
                lt = rcp.tile([HD, QC], F32, tag="ln", name=f"ln{jq}_{t}_{h}")
                rt = rcp.tile([HD, QC], F32, tag="rc", name=f"rc{jq}_{t}_{h}")
                nc.scalar.activation(lt[:], po[h][HD : 2 * HD, :], LN)
                nc.scalar.activation(rt[:], lt[:], EXP, scale=-1.0)
                led["act"] += 2 * (QC + 352) / 1.2
                rts.append(rt)
            if jq == 3 and t == pair_order[0]:
                # release the reserved oproj(2) chains (deps long done) so
                # the boundary pump below has work to cover the normalize
                fillers.append(gen_oproj(2))
            # pump fillers BEFORE the muls: their DVE copies then sit ahead
            # of the ACT-blocked muls in the DVE FIFO and drain freely
            pump()
            for _ in range(6):
                pump_one()
            for h in range(2):
                nc.vector.tensor_mul(
                    ats[t][h * HD : (h + 1) * HD, q0 : q0 + QC],
                    po[h][0:HD, :],
                    rts[h][:],
                )
            if jq == 3 and t == pair_order[0]:
                # fc1 singles depend on the normalize just emitted; queue them
                # after it so the pump can't pull them into a stall
                fillers.append(gen_oproj3_fc(1))

    # ---- program ----
    # proj(0) emitted eagerly
    drain_gen(gen_qk(0, wq_s, qts, "q"))
    drain_gen(gen_qk(0, wk_s, kts, "k"))
    drain_gen(gen_v(0))

    filler_plan = {
        0: lambda: [
            gen_qk(1, wq_s, qts, "q"),
            gen_qk(1, wk_s, kts, "k"),
            make_v_tracked(1),
        ],
        1: lambda: [
            gen_qk(2, wq_s, qts, "q"),
            gen_qk(2, wk_s, kts, "k"),
            make_v_tracked(2),
        ],
        2: lambda: [
            gen_qk(3, wq_s, qts, "q"),
            gen_qk(3, wk_s, kts, "k"),
            gen_oproj(0),
        ],
        3: lambda: [make_v_tracked(3), gen_oproj(1)],
    }

    for jq in range(4):
        if jq < 3:  # prefetch next chunk's x
            xall[jq + 1] = xpool.tile([KB, 8 * QC], BF16, tag="x", name=f"x{jq + 1}")
            nc.sync.dma_start(
                xall[jq + 1][:].rearrange("p (h c) -> p h c", h=8), xt_d[jq + 1]
            )
        led["pe"] = led["act"] = 0.0
        for g in filler_plan[jq]():
            fillers.append(g)
        attention(jq)
        drain_all()


def _build():
    if "nc" in _CACHE:
        return _CACHE["nc"]
    nc = bacc.Bacc(
        "TRN2", target_bir_lowering=False, debug=False, num_devices=N_CORES
    )
    with tile.TileContext(nc) as tc:
        with ExitStack() as ctx:
            tc._emit_ctx = ctx
            _emit(tc)
    nc.compile()
    _CACHE["nc"] = nc
    return nc


def _numpy_fallback(q, attention_mask, Wq, Wk, Wv, Wo):
    import math

    b, s, _ = q.shape
    causal = np.tril(np.ones((s, s), bool))
    valid = attention_mask != 0
    mask = causal[None] & valid[:, :, None] & valid[:, None, :]
    mask = mask[:, None]
    out = np.zeros((b, s, H), np.float32)
    for bi in range(b):
        x = q[bi]
        nh = x.shape[1] // HD
        qh = (x @ Wq.T).reshape(s, nh, HD).transpose(1, 0, 2)
        kh = (x @ Wk.T).reshape(s, nh, HD).transpose(1, 0, 2)
        vh = (x @ Wv.T).reshape(s, nh, HD).transpose(1, 0, 2)
        sc = np.einsum("hqd,hkd->hqk", qh, kh) / math.sqrt(HD)
        sc = np.where(mask[bi], sc, np.float32(-1e6))
        sc = sc - sc.max(-1, keepdims=True)
        e = np.exp(sc)
        p = e / e.sum(-1, keepdims=True)
        p = np.where(mask[bi], p, np.float32(0.0))
        o = np.einsum("hqk,hkd->hqd", p, vh).transpose(1, 0, 2).reshape(s, -1)
        out[bi] = o @ Wo.T
    return out


def _run(q, attention_mask, Wq, Wk, Wv, Wo, trace=False, **trace_kwargs):
    q = np.ascontiguousarray(np.asarray(q, dtype=np.float32))
    Wq = np.asarray(Wq, dtype=np.float32)
    Wk = np.asarray(Wk, dtype=np.float32)
    Wv = np.asarray(Wv, dtype=np.float32)
    Wo = np.asarray(Wo, dtype=np.float32)
    am = np.asarray(attention_mask)
    if q.shape != (B, S, H) or not np.all(am != 0):
        return _numpy_fallback(q, am, Wq, Wk, Wv, Wo), None

    bf = ml_dtypes.bfloat16
    idx = np.arange(KB)
    mtri = (idx[:, None] <= idx[None, :]).astype(bf)

    # x^T pre-tiled per batch: [4 chunks][8 hc][128][512]
    xts = []
    for b in range(B):
        xt = np.ascontiguousarray(
            q[b].T.reshape(8, KB, 4, QC).transpose(2, 1, 0, 3).astype(bf)
        )
        xts.append(xt)

    in_maps = []
    for c in range(N_CORES):
        b, g = c // 4, c % 4
        fs = slice(F * g, F * (g + 1))
        in_maps.append(
            {
                "xt": xts[b],
                "wq": np.ascontiguousarray(Wq[fs, :].T.reshape(8, KB, F).transpose(1, 0, 2).astype(bf)),
                "wk": np.ascontiguousarray(Wk[fs, :].T.reshape(8, KB, F).transpose(1, 0, 2).astype(bf)),
                "wv": np.ascontiguousarray(Wv[fs, :].T.reshape(8, KB, F).transpose(1, 0, 2).astype(bf)),
                "wo": np.ascontiguousarray(
                    Wo[:, fs].T.reshape(2, KB, 2 * QC).transpose(1, 0, 2).astype(bf)
                ),
                "mtri": mtri,
            }
        )

    nc = _build()
    res = bass_utils.run_bass_kernel_spmd(
        nc, in_maps, core_ids=list(range(N_CORES)), trace=trace, **trace_kwargs
    )
    # out: [16 qb][2 oc][128][512] bf16 (+ chunk-3 fc1 partial) -> [2048, 1024]
    outs = []
    for r in res.results:
        o = np.asarray(r["out"]).astype(np.float32).reshape(16, 2, KB, QC)
        o2 = np.asarray(r["out2"]).astype(np.float32).reshape(4, 2, KB, QC)
        o[12:16] += o2
        outs.append(o.transpose(0, 2, 1, 3).reshape(S, H))
    full = np.empty((B, S, H), np.float32)
    for b in range(B):
        full[b] = outs[4 * b] + outs[4 * b + 1] + outs[4 * b + 2] + outs[4 * b + 3]
    return full, res


def kernel(q, attention_mask, Wq, Wk, Wv, Wo):
    out, _ = _run(q, attention_mask, Wq, Wk, Wv, Wo)
    return out


# revision 27
# speedup vs baseline: 1.0085x; 1.0085x over previous
"""Multi-head attention (bsz=2, seq=2048, hidden=1024, heads=16) on 8 TRN2 cores.

Sharding: core c = 4*b + g handles batch b and heads [4g, 4g+4).
Each core computes Q/K/V projections for its 4 heads, causal softmax
attention, and a partial output projection over its 256 features; the host
sums the 4 per-batch partials.

Redesign vs the 322us baseline (which ran the PE at cold clock):
- bf16 operands everywhere (same 1 cyc/row as f32r, half the DMA/SBUF/DVE
  cost, FWL-eligible weight loads); fp32 PSUM accumulation throughout.
- Host pre-tiles every DRAM tensor partition-major so each load is one large
  partition-contiguous DMA (the first x chunk is split per-slice so the
  first projection chain starts as slices land).
- Dummy warmup matmuls on the mask tile bridge the initial DMA window so the
  HAM clock gate reaches full clock before real work; the emission never
  leaves PE-idle gaps >3.4us, so it never re-throttles.
- Scores for a head PAIR run as two concurrent row-tiled matmuls (head-dim
  64 contraction at base partitions 0/64) into one 2-bank PSUM tile, so the
  exp covers [128, 2x512] in a single ACT instruction (amortizes the
  352-cycle ACT overhead).
- Softmax 1/denominator as exp(-ln(d)) on the scalar engine (the DVE
  reciprocal is ~6 cyc/elem and blocks the DVE FIFO); a pre-placed
  InstLoadActFuncSet pins the combined natural_log_exp_and_others table so
  the compiler never thrashes table sets.
- Software-pipelined emission: PV lags scores by one block, and projection /
  output-projection matmuls of neighboring chunks are pumped as fillers
  between attention blocks, paced by a pe-vs-act ns ledger, so the strict
  FIFO PE queue never waits on the scalar engine. The last chunk runs its
  head pairs in reverse, splits its O-projection per pair (host sums the
  extra partial), and normalizes per q-half so the tail pipelines.
"""

import sys

sys.path.insert(0, "/opt/trn_rl_repo")

from collections import deque
from contextlib import ExitStack

import numpy as np
import ml_dtypes

import concourse.tile as tile
from concourse import bacc, bass_utils, mybir

B, S, H = 2, 2048, 1024
NHC = 4  # heads per core
HD = 64  # head dim
F = NHC * HD  # features per core (256)
N_CORES = 8
QC = 512  # query-chunk width
KB = 128  # key-block size
SCALE = 1.0 / 8.0  # 1/sqrt(HD)
NWARM = 26

F32 = mybir.dt.float32
F32R = mybir.dt.float32r
BF16 = mybir.dt.bfloat16
EXP = mybir.ActivationFunctionType.Exp
LN = mybir.ActivationFunctionType.Ln

_CACHE = {}


def _emit(tc):
    nc = tc.nc
    xt_d = nc.dram_tensor("xt", [4, KB, 8, QC], BF16, kind="ExternalInput").ap()
    wq_d = nc.dram_tensor("wq", [KB, 8, F], BF16, kind="ExternalInput").ap()
    wk_d = nc.dram_tensor("wk", [KB, 8, F], BF16, kind="ExternalInput").ap()
    wv_d = nc.dram_tensor("wv", [KB, 8, F], BF16, kind="ExternalInput").ap()
    wo_d = nc.dram_tensor("wo", [KB, 2, 2 * QC], BF16, kind="ExternalInput").ap()
    mtri_d = nc.dram_tensor("mtri", [KB, KB], BF16, kind="ExternalInput").ap()
    out_d = nc.dram_tensor("out", [16, 2, KB, QC], BF16, kind="ExternalOutput").ap()
    # chunk-3 O-proj is split per head-pair so its fc1 half can run early;
    # the host sums this extra partial into out[12:16]
    out2_d = nc.dram_tensor("out2", [4, 2, KB, QC], BF16, kind="ExternalOutput").ap()

    ctx = tc._emit_ctx
    const = ctx.enter_context(tc.tile_pool(name="const", bufs=1))
    persist = ctx.enter_context(tc.tile_pool(name="persist", bufs=1))
    xpool = ctx.enter_context(tc.tile_pool(name="xp", bufs=2))
    ptp = ctx.enter_context(tc.tile_pool(name="ptp", bufs=4))
    rcp = ctx.enter_context(tc.tile_pool(name="rcp", bufs=2))
    ost = ctx.enter_context(tc.tile_pool(name="ost", bufs=6))
    ps_st = ctx.enter_context(tc.tile_pool(name="psst", bufs=2, space="PSUM"))
    ps_po = ctx.enter_context(tc.tile_pool(name="pspo", bufs=2, space="PSUM"))
    ps_pj = ctx.enter_context(tc.tile_pool(name="pspj", bufs=2, space="PSUM"))

    # ---- persistent SBUF tiles ----
    wq_s = persist.tile([KB, 8 * F], BF16, tag="wq_s")
    wk_s = persist.tile([KB, 8 * F], BF16, tag="wk_s")
    wv_s = persist.tile([KB, 8 * F], BF16, tag="wv_s")
    wo_s = persist.tile([KB, 4 * QC], BF16, tag="wo_s")
    mtri = const.tile([KB, KB], BF16, tag="mtri")
    qts = [persist.tile([KB, S], BF16, tag=f"qt{i}", name=f"qt{i}") for i in range(2)]
    kts = [persist.tile([KB, S], BF16, tag=f"kt{i}", name=f"kt{i}") for i in range(2)]
    ats = [persist.tile([KB, S], BF16, tag=f"at{i}", name=f"at{i}") for i in range(2)]
    # V_aug per k-block: per head [V_h (64) | ones (64)] -> [128, 512]
    vts = [
        persist.tile([KB, NHC * 2 * HD], BF16, tag=f"vt{i}", name=f"vt{i}")
        for i in range(16)
    ]

    # ---- initial DMAs, priority order (Q-proj needs wq+x0 first) ----
    nc.sync.dma_start(mtri[:], mtri_d[:])
    nc.sync.dma_start(wq_s[:].rearrange("p (h c) -> p h c", h=8), wq_d[:])
    xall = [None] * 4
    xall[0] = xpool.tile([KB, 8 * QC], BF16, tag="x", name="x0")
    for hc in range(8):
        nc.sync.dma_start(
            xall[0][:, hc * QC : (hc + 1) * QC], xt_d[0, :, hc]
        )
    nc.sync.dma_start(wk_s[:].rearrange("p (h c) -> p h c", h=8), wk_d[:])
    nc.sync.dma_start(wv_s[:].rearrange("p (h c) -> p h c", h=8), wv_d[:])
    nc.sync.dma_start(wo_s[:].rearrange("p (h c) -> p h c", h=2), wo_d[:])

    # ---- memsets (DVE) ----
    for rc in range(16):
        v4 = vts[rc][:].rearrange("p (h d) -> p h d", h=NHC)
        nc.vector.memset(v4[:, :, HD : 2 * HD], 1.0)

    # ---- PE warmup (bridges the initial DMA window; gets HAM warm) ----
    for i in range(NWARM):
        ps = ps_pj.tile([KB, QC], F32, tag="pj", name=f"warm{i}")
        nc.tensor.matmul(ps[:, :KB], mtri[:], mtri[:], start=True, stop=True)
    # pre-place the combined ln+exp table set (natural_log_exp_and_others,
    # id 6) so the fixpoint never thrashes between exp_and_others and
    # natural_log at every softmax normalize
    nc.scalar.add_instruction(
        mybir.InstLoadActFuncSet(
            name=nc.get_next_instruction_name(), act_func_set_id=6, ins=[], outs=[]
        )
    )
    warm_pt = const.tile([1, 4], BF16, tag="warm_pt")
    nc.scalar.activation(warm_pt[:], mtri[0:1, 0:4], EXP, scale=SCALE)

    # ---- pacing ledger for filler pumping ----
    led = {"pe": 0.0, "act": 0.0}
    fillers = deque()
    vgen = {}  # rc -> generator writing vts[rc], for targeted force-drain

    def pump_one():
        while fillers:
            gen = fillers[0]
            try:
                led["pe"] += next(gen)
                return True
            except StopIteration:
                fillers.popleft()
        return False

    def pump():
        while led["act"] > led["pe"]:
            if not pump_one():
                return

    def drain_all():
        while pump_one():
            pass

    def drain_gen(gen):
        for _ in gen:
            pass

    # ---- projection chain generators (each yield = one PE matmul emitted) ----
    def gen_qk(jq, w_s, dst, what):
        xa = xall[jq]
        q0 = jq * QC
        for fc in range(2):
            ps = ps_pj.tile([KB, QC], F32, tag="pj", name=f"p{what}{jq}_{fc}")
            for hc in range(8):
                nc.tensor.matmul(
                    ps[:],
                    w_s[:, hc * F + fc * KB : hc * F + (fc + 1) * KB],
                    xa[:, hc * QC : (hc + 1) * QC],
                    start=(hc == 0),
                    stop=(hc == 7),
                )
                yield 223
            nc.vector.tensor_copy(dst[fc][:, q0 : q0 + QC], ps[:])

    def gen_v(jq):
        xa = xall[jq]
        for sub in range(4):
            rc = 4 * jq + sub
            ps = ps_pj.tile([KB, QC], F32, tag="pj", name=f"pv{rc}")
            for hc in range(8):
                nc.tensor.matmul(
                    ps[:, :F],
                    xa[:, hc * QC + sub * KB : hc * QC + (sub + 1) * KB],
                    wv_s[:, hc * F : (hc + 1) * F],
                    start=(hc == 0),
                    stop=(hc == 7),
                )
                yield 117
            v4 = vts[rc][:].rearrange("p (h d) -> p h d", h=NHC)
            nc.vector.tensor_copy(
                v4[:, :, 0:HD], ps[:, :F].rearrange("p (h d) -> p h d", h=NHC)
            )
            vgen.pop(rc, None)

    def make_v_tracked(jq):
        # split per-sub and register eagerly so pv_block can force-drain
        # exactly the tile it needs even before the pump reaches it
        gens = []
        for sub in range(4):
            rc = 4 * jq + sub
            g = _gen_v_one(jq, sub, rc)
            vgen[rc] = g
            gens.append(g)

        def runner():
            for g in gens:
                yield from g

        return runner()

    def _gen_v_one(jq, sub, rc):
        xa = xall[jq]
        ps = ps_pj.tile([KB, QC], F32, tag="pj", name=f"pv{rc}")
        for hc in range(8):
            nc.tensor.matmul(
                ps[:, :F],
                xa[:, hc * QC + sub * KB : hc * QC + (sub + 1) * KB],
                wv_s[:, hc * F : (hc + 1) * F],
                start=(hc == 0),
                stop=(hc == 7),
            )
            yield 117
        v4 = vts[rc][:].rearrange("p (h d) -> p h d", h=NHC)
        nc.vector.tensor_copy(
            v4[:, :, 0:HD], ps[:, :F].rearrange("p (h d) -> p h d", h=NHC)
        )
        vgen.pop(rc, None)

    def gen_oproj(jq):
        for sub in range(4):
            qb = 4 * jq + sub
            for oc in range(2):
                ps = ps_pj.tile([KB, QC], F32, tag="pj", name=f"po{qb}_{oc}")
                for fc in range(2):
                    nc.tensor.matmul(
                        ps[:],
                        ats[fc][:, qb * KB : (qb + 1) * KB],
                        wo_s[:, fc * 2 * QC + oc * QC : fc * 2 * QC + (oc + 1) * QC],
                        start=(fc == 0),
                        stop=(fc == 1),
                    )
                    yield 223
                o = ost.tile([KB, QC], BF16, tag="ost", name=f"os{qb}_{oc}")
                nc.vector.tensor_copy(o[:], ps[:])
                nc.sync.dma_start(out_d[qb, oc], o[:])

    def gen_oproj3_fc(fc):
        # chunk-3 O-proj, one head-pair at a time: single-MM partials so the
        # fc1 half runs as soon as pair 1 is normalized (host sums partials)
        for sub in range(4):
            qb = 12 + sub
            for oc in range(2):
                ps = ps_pj.tile([KB, QC], F32, tag="pj", name=f"po3{fc}_{qb}_{oc}")
                nc.tensor.matmul(
                    ps[:],
                    ats[fc][:, qb * KB : (qb + 1) * KB],
                    wo_s[:, fc * 2 * QC + oc * QC : fc * 2 * QC + (oc + 1) * QC],
                    start=True,
                    stop=True,
                )
                yield 223
                o = ost.tile([KB, QC], BF16, tag="ost", name=f"o3{fc}_{qb}_{oc}")
                nc.vector.tensor_copy(o[:], ps[:])
                if fc == 0:
                    nc.sync.dma_start(out_d[qb, oc], o[:])
                else:
                    nc.sync.dma_start(out2_d[sub, oc], o[:])

    # ---- attention for one chunk (both head pairs), filler-interleaved ----
    def attention(jq):
        q0 = jq * QC
        nkb = 4 * jq + 4
        # last chunk: pair 1 first so its O-proj half overlaps pair 0
        pair_order = (1, 0) if jq == 3 else (0, 1)
        for t in pair_order:
            po = [
                ps_po.tile([KB, QC], F32, tag="po", name=f"pp{jq}_{t}_{h}")
                for h in range(2)
            ]
            pts = {}

            def s_block(ik):
                r = ik - 4 * jq
                qk = 0 if r <= 0 else r * KB
                ex = qk
                st = ps_st.tile([KB, 2 * QC], F32, tag="st", name=f"st{jq}_{t}_{ik}")
                nc.tensor.matmul(
                    st[:, qk:QC],
                    kts[t][0:HD, ik * KB : (ik + 1) * KB],
                    qts[t][0:HD, q0 + qk : q0 + QC],
                    start=True,
                    stop=True,
                )
                nc.tensor.matmul(
                    st[:, QC + qk : 2 * QC],
                    kts[t][HD : 2 * HD, ik * KB : (ik + 1) * KB],
                    qts[t][HD : 2 * HD, q0 + qk : q0 + QC],
                    start=True,
                    stop=True,
                )
                led["pe"] += (QC - qk) / 2.4 + 20
                pt = ptp.tile([KB, 2 * QC], BF16, tag="pt", name=f"pt{jq}_{t}_{ik}")
                stv = st[:].rearrange("p (h c) -> p h c", h=2)
                ptv = pt[:].rearrange("p (h c) -> p h c", h=2)
                nc.scalar.activation(
                    ptv[:, :, ex:QC], stv[:, :, ex:QC], EXP, scale=SCALE
                )
                led["act"] += (2 * (QC - ex) + 352) / 1.2
                if r >= 0:
                    trv = ptv[:, :, r * KB : (r + 1) * KB]
                    mb = mtri[:].rearrange("p c -> p () c").broadcast_to(
                        (KB, 2, KB)
                    )
                    nc.vector.tensor_mul(trv, trv, mb)
                pts[ik] = (pt, qk)

            def pv_block(ik):
                if ik in vgen:  # guarantee vts[ik] is emitted before its reader
                    drain_gen(vgen.pop(ik))
                pt, qk = pts.pop(ik)
                for h in range(2):
                    hh = 2 * t + h
                    nc.tensor.matmul(
                        po[h][:, qk:QC],
                        vts[ik][:, hh * 2 * HD : (hh + 1) * 2 * HD],
                        pt[:, h * QC + qk : h * QC + QC],
                        start=(ik == 0),
                        stop=(ik == nkb - 1),
                    )
                led["pe"] += 2 * (QC - qk) / 2.4 + 40

            for ik in range(nkb):
                s_block(ik)
                if ik >= 1:
                    pv_block(ik - 1)
                pump()
            pv_block(nkb - 1)
            # normalize: partitions 64:128 of po hold the denominator.
            # 1/d as exp(-ln(d)) on the scalar engine (pipelined; the DVE
            # iterative reciprocal is ~6 cyc/elem and stalls the pipeline).
            # ln+exp share one table set (natural_log_exp_and_others).
            if jq == 3 and t == pair_order[1]:
                # final pair: normalize per q-half, interleaving the fc0
                # O-proj singles so the tail chain pipelines
                g0 = gen_oproj3_fc(0)
                for half in range(2):
                    c0, c1 = half * 256, (half + 1) * 256
                    for h in range(2):
                        lt = rcp.tile(
                            [HD, 256], F32, tag="lnh", name=f"lnh{half}_{h}"
                        )
                        rt = rcp.tile(
                            [HD, 256], F32, tag="rch", name=f"rch{half}_{h}"
                        )
                        nc.scalar.activation(lt[:], po[h][HD : 2 * HD, c0:c1], LN)
                        nc.scalar.activation(rt[:], lt[:], EXP, scale=-1.0)
                        nc.vector.tensor_mul(
                            ats[t][h * HD : (h + 1) * HD, q0 + c0 : q0 + c1],
                            po[h][0:HD, c0:c1],
                            rt[:],
                        )
                    for _ in range(4):  # subs {0,1} then {2,3}, 2 oc each
                        try:
                            next(g0)
                        except StopIteration:
                            break
                drain_gen(g0)
                return
            rts = []
            for h in range(2):
                lt = rcp.tile([HD, QC], F32, tag="ln", name=f"ln{jq}_{t}_{h}")
                rt = rcp.tile([HD, QC], F32, tag="rc", name=f"rc{jq}_{t}_{h}")
                nc.scalar.activation(lt[:], po[h][HD : 2 * HD, :], LN)
                nc.scalar.activation(rt[:], lt[:], EXP, scale=-1.0)
                led["act"] += 2 * (QC + 352) / 1.2
                rts.append(rt)
            if jq == 3 and t == pair_order[0]:
                # release the reserved oproj(2) chains (deps long done) and
                # force 10 of their matmuls in right here: they sit directly
                # behind the last PV in the PE FIFO and run while the ACT
                # normalize chain (ln/exp x2) drains
                fillers.append(gen_oproj(2))
                for _ in range(10):
                    pump_one()
            # pump fillers BEFORE the muls: their DVE copies then sit ahead
            # of the ACT-blocked muls in the DVE FIFO and drain freely
            pump()
            for _ in range(6):
                pump_one()
            for h in range(2):
                nc.vector.tensor_mul(
                    ats[t][h * HD : (h + 1) * HD, q0 : q0 + QC],
                    po[h][0:HD, :],
                    rts[h][:],
                )
            if jq == 3 and t == pair_order[0]:
                # fc1 singles depend on the normalize just emitted; queue them
                # after it so the pump can't pull them into a stall
                fillers.append(gen_oproj3_fc(1))

    # ---- program ----
    # proj(0) emitted eagerly
    drain_gen(gen_qk(0, wq_s, qts, "q"))
    drain_gen(gen_qk(0, wk_s, kts, "k"))
    drain_gen(gen_v(0))

    filler_plan = {
        0: lambda: [
            gen_qk(1, wq_s, qts, "q"),
            gen_qk(1, wk_s, kts, "k"),
            make_v_tracked(1),
        ],
        1: lambda: [
            gen_qk(2, wq_s, qts, "q"),
            gen_qk(2, wk_s, kts, "k"),
            make_v_tracked(2),
        ],
        2: lambda: [
            gen_qk(3, wq_s, qts, "q"),
            gen_qk(3, wk_s, kts, "k"),
            gen_oproj(0),
        ],
        3: lambda: [make_v_tracked(3), gen_oproj(1)],
    }

    for jq in range(4):
        if jq < 3:  # prefetch next chunk's x
            xall[jq + 1] = xpool.tile([KB, 8 * QC], BF16, tag="x", name=f"x{jq + 1}")
            nc.sync.dma_start(
                xall[jq + 1][:].rearrange("p (h c) -> p h c", h=8), xt_d[jq + 1]
            )
        led["pe"] = led["act"] = 0.0
        for g in filler_plan[jq]():
            fillers.append(g)
        attention(jq)
        drain_all()


def _build():
    if "nc" in _CACHE:
        return _CACHE["nc"]
    nc = bacc.Bacc(
        "TRN2", target_bir_lowering=False, debug=False, num_devices=N_CORES
    )
    with tile.TileContext(nc) as tc:
        with ExitStack() as ctx:
            tc._emit_ctx = ctx
            _emit(tc)
    nc.compile()
    _CACHE["nc"] = nc
    return nc


def _numpy_fallback(q, attention_mask, Wq, Wk, Wv, Wo):
    import math

    b, s, _ = q.shape
    causal = np.tril(np.ones((s, s), bool))
    valid = attention_mask != 0
    mask = causal[None] & valid[:, :, None] & valid[:, None, :]
    mask = mask[:, None]
    out = np.zeros((b, s, H), np.float32)
    for bi in range(b):
        x = q[bi]
        nh = x.shape[1] // HD
        qh = (x @ Wq.T).reshape(s, nh, HD).transpose(1, 0, 2)
        kh = (x @ Wk.T).reshape(s, nh, HD).transpose(1, 0, 2)
        vh = (x @ Wv.T).reshape(s, nh, HD).transpose(1, 0, 2)
        sc = np.einsum("hqd,hkd->hqk", qh, kh) / math.sqrt(HD)
        sc = np.where(mask[bi], sc, np.float32(-1e6))
        sc = sc - sc.max(-1, keepdims=True)
        e = np.exp(sc)
        p = e / e.sum(-1, keepdims=True)
        p = np.where(mask[bi], p, np.float32(0.0))
        o = np.einsum("hqk,hkd->hqd", p, vh).transpose(1, 0, 2).reshape(s, -1)
        out[bi] = o @ Wo.T
    return out


def _run(q, attention_mask, Wq, Wk, Wv, Wo, trace=False, **trace_kwargs):
    q = np.ascontiguousarray(np.asarray(q, dtype=np.float32))
    Wq = np.asarray(Wq, dtype=np.float32)
    Wk = np.asarray(Wk, dtype=np.float32)
    Wv = np.asarray(Wv, dtype=np.float32)
    Wo = np.asarray(Wo, dtype=np.float32)
    am = np.asarray(attention_mask)
    if q.shape != (B, S, H) or not np.all(am != 0):
        return _numpy_fallback(q, am, Wq, Wk, Wv, Wo), None

    bf = ml_dtypes.bfloat16
    idx = np.arange(KB)
    mtri = (idx[:, None] <= idx[None, :]).astype(bf)

    # x^T pre-tiled per batch: [4 chunks][8 hc][128][512]
    xts = []
    for b in range(B):
        xt = np.ascontiguousarray(
            q[b].T.reshape(8, KB, 4, QC).transpose(2, 1, 0, 3).astype(bf)
        )
        xts.append(xt)

    in_maps = []
    for c in range(N_CORES):
        b, g = c // 4, c % 4
        fs = slice(F * g, F * (g + 1))
        in_maps.append(
            {
                "xt": xts[b],
                "wq": np.ascontiguousarray(Wq[fs, :].T.reshape(8, KB, F).transpose(1, 0, 2).astype(bf)),
                "wk": np.ascontiguousarray(Wk[fs, :].T.reshape(8, KB, F).transpose(1, 0, 2).astype(bf)),
                "wv": np.ascontiguousarray(Wv[fs, :].T.reshape(8, KB, F).transpose(1, 0, 2).astype(bf)),
                "wo": np.ascontiguousarray(
                    Wo[:, fs].T.reshape(2, KB, 2 * QC).transpose(1, 0, 2).astype(bf)
                ),
                "mtri": mtri,
            }
        )

    nc = _build()
    res = bass_utils.run_bass_kernel_spmd(
        nc, in_maps, core_ids=list(range(N_CORES)), trace=trace, **trace_kwargs
    )
    # out: [16 qb][2 oc][128][512] bf16 (+ chunk-3 fc1 partial) -> [2048, 1024]
    outs = []
    for r in res.results:
        o = np.asarray(r["out"]).astype(np.float32).reshape(16, 2, KB, QC)
        o2 = np.asarray(r["out2"]).astype(np.float32).reshape(4, 2, KB, QC)
        o[12:16] += o2
        outs.append(o.transpose(0, 2, 1, 3).reshape(S, H))
    full = np.empty((B, S, H), np.float32)
    for b in range(B):
        full[b] = outs[4 * b] + outs[4 * b + 1] + outs[4 * b + 2] + outs[4 * b + 3]
    return full, res


def kernel(q, attention_mask, Wq, Wk, Wv, Wo):
    out, _ = _run(q, attention_mask, Wq, Wk, Wv, Wo)
    return out
